# revision 20
# baseline (speedup 1.0000x reference)
"""Trainium2 Bass kernel for causal multi-head attention with RoPE.

Problem: B=2, S=2048, DIM=1024, 16 heads, head_dim=64.
  q = x @ Wq.T ; k = x @ Wk.T ; v = x @ Wv.T        (torch Linear convention)
  q, k = rope(q), rope(k)                            (Llama interleaved pairs)
  y = softmax(causal(q k^T / 8)) v @ Wo.T

Sharding (8 cores): data-parallel over batch (2) x tensor-parallel over
head groups (4 heads per core).  Wq/Wk/Wv row-sharded, Wo column-sharded;
the host sums the 4 partial outputs per batch.

Fused single-stream schedule (v2): projections, attention, and the output
projection are interleaved at fine grain so the PE never waits on the
ScalarE exp stream and ScalarE starts exp'ing ~12us into the kernel
instead of ~70us:

  P(t): Q/K/V projections for one 512-token block (PE) + RoPE (ACT cast,
        DVE mul/rot/add) + V copy to the PV layout (ACT).
  A(q): causal attention for one 512-query block: per 128-k-block j,
        ST (PE, scores^T in PSUM, 2 heads packed in one [128,2,512] tile)
        -> exp (one ScalarE instr over both heads) -> PV accumulate
        (PE, [V|1]-stationary, gives the softmax denominator for free).
        Diagonal-block PV chunks deferred past the DVE causal-mask mul.
        Normalization decoupled from PSUM: OB is copied to SBUF (DVE)
        so the PSUM bank recycles immediately; reciprocal + GpSimd
        partition-broadcast + DVE mul happen off the critical path.
  O(q): output projection of one finished 512-query block + DMA out.

  Emission order (PE executes in order; fillers absorb exp latency):
    warmup-mm, P(0), A(0)[P(1),P(2) interleaved], A(1)[P(3)],
    A(2)[O(0)], A(3)[O(1),O(2)], O(3)

All matmul operands fp16 (accumulation fp32), scores exp'd without max
subtraction (they are O(1) here).  Host bakes: RoPE pair de-interleave
into the W rows (rope swap = +-32 partition copy), cos/sin tables with
signs, causal 0/1 mask, and fp16 transposed layouts.
"""

import os
import sys

sys.path.insert(0, "/opt/trn_rl_repo")

from collections import deque

import numpy as np

import concourse.bass as bass
import concourse.mybir as mybir
import concourse.tile as tile
from concourse import bacc
from concourse.bass_utils import run_bass_kernel_spmd

F16 = mybir.dt.float16
F32 = mybir.dt.float32

DIM = 1024
NUM_HEADS = 16
HEAD_DIM = 64
B = 2
GROUPS = 4                   # head groups (tensor parallel)
HPG = NUM_HEADS // GROUPS    # heads per group = 4
FG = HPG * HEAD_DIM          # features per group = 256
THETA = 10000.0


DEBUG_TAPS = False


def build_program(S=2048):
    from contextlib import ExitStack

    nc = bacc.Bacc(None, target_bir_lowering=False)
    NT = S // 128                 # 128-token blocks (16)
    QB = 512                      # query/token macro-block
    NQB = S // QB                 # 4

    xt_d = nc.declare_dram_parameter("xt", [DIM, S], F16, isOutput=False)
    wq_d = nc.declare_dram_parameter("wqt", [DIM, FG], F16, isOutput=False)
    wk_d = nc.declare_dram_parameter("wkt", [DIM, FG], F16, isOutput=False)
    wv_d = nc.declare_dram_parameter("wvt", [DIM, FG], F16, isOutput=False)
    wo_d = nc.declare_dram_parameter("wot", [FG, DIM], F16, isOutput=False)
    cos_d = nc.declare_dram_parameter("cos", [128, S], F16, isOutput=False)
    sin_d = nc.declare_dram_parameter("sins", [128, S], F16, isOutput=False)
    mask_d = nc.declare_dram_parameter("mask", [128, 128], F16, isOutput=False)
    # fp16 partial output; host upcasts to fp32 before summing the 4 partials
    yt_d = nc.declare_dram_parameter("yt", [DIM, S], F16, isOutput=True)

    Exp = mybir.ActivationFunctionType.Exp

    with tile.TileContext(nc) as tc:
        with ExitStack() as ctx:
            consts = ctx.enter_context(tc.tile_pool(name="consts", bufs=1))
            ps = ctx.enter_context(tc.tile_pool(name="ps", bufs=2, space="PSUM"))
            ptp = ctx.enter_context(tc.tile_pool(name="ptp", bufs=8))
            rp = ctx.enter_context(tc.tile_pool(name="rope", bufs=3))
            npl = ctx.enter_context(tc.tile_pool(name="norm", bufs=4))
            yp = ctx.enter_context(tc.tile_pool(name="ysb", bufs=3))

            wq_sb = consts.tile([128, 8, FG], F16)
            wk_sb = consts.tile([128, 8, FG], F16)
            wv_sb = consts.tile([128, 8, FG], F16)
            wo_sb = consts.tile([128, 2, DIM], F16)
            xt_sb = consts.tile([128, 8, S], F16)
            cos_t = consts.tile([128, S], F16)
            sin_t = consts.tile([128, S], F16)
            mask_t = consts.tile([128, 128], F16)
            qt_sb = consts.tile([128, 2, S], F16)
            kt_sb = consts.tile([128, 2, S], F16)
            zt_sb = consts.tile([128, 2, S], F16)
            vaug = consts.tile([128, NT, HPG * 65], F16)
            warm_sb = consts.tile([128, 512], F16)

            nc.vector.memset(warm_sb[:], 1.0)
            nc.vector.memset(vaug[:], 1.0)

            # ---------- input DMA, need-ordered ----------
            xt_r = xt_d[:].rearrange("(c p) t -> p c t", p=128)

            def dma_xt(tb):
                for k in range(8):
                    nc.sync.dma_start(
                        xt_sb[:, k, tb * QB:(tb + 1) * QB],
                        xt_r[:, k, tb * QB:(tb + 1) * QB],
                    )

            nc.sync.dma_start(wk_sb[:], wk_d[:].rearrange("(c p) f -> p c f", p=128))
            dma_xt(0)
            nc.sync.dma_start(wq_sb[:], wq_d[:].rearrange("(c p) f -> p c f", p=128))
            nc.sync.dma_start(wv_sb[:], wv_d[:].rearrange("(c p) f -> p c f", p=128))
            nc.sync.dma_start(cos_t[:], cos_d[:])
            nc.sync.dma_start(sin_t[:], sin_d[:])
            dma_xt(1)
            nc.sync.dma_start(mask_t[:], mask_d[:])
            dma_xt(2)
            dma_xt(3)
            nc.sync.dma_start(wo_sb[:], wo_d[:].rearrange("(c p) d -> p c d", p=128))

            # ---------- PE p-state warmup (results discarded) ----------
            for i in range(3):
                pw = ps.tile([128, 512], F32, tag="pp", name=f"warm{i}")
                nc.tensor.matmul(
                    pw[:], lhsT=warm_sb[:, 0:128], rhs=warm_sb[:],
                    start=True, stop=True,
                )

            # ---------- unit emitters ----------
            def rope_tail(psq, dest, tb, c):
                lo = tb * QB
                qc = rp.tile([128, QB], F16, tag="qc")
                nc.scalar.copy(qc[:], psq[:])  # fp32->fp16 cast on ACT
                t1 = rp.tile([128, QB], F16, tag="t1")
                nc.vector.tensor_mul(t1[:], qc[:], cos_t[:, lo:lo + QB])
                rot = rp.tile([128, QB], F16, tag="rot")
                for qq in range(4):
                    srcp = (qq ^ 1) * 32
                    nc.vector.tensor_copy(
                        rot[qq * 32:(qq + 1) * 32, :], qc[srcp:srcp + 32, :]
                    )
                t2 = rp.tile([128, QB], F16, tag="t2")
                nc.vector.tensor_mul(t2[:], rot[:], sin_t[:, lo:lo + QB])
                nc.vector.tensor_add(dest[:, c, lo:lo + QB], t1[:], t2[:])

            def proj_qk_streamed(wsb, dest, tb):
                """k-outer / c-inner: first matmul needs only the first xt
                chunk, so the head of the kernel is not DMA-bound."""
                lo = tb * QB
                psqs = [
                    ps.tile([128, QB], F32, tag="pp", name=f"psqs_{tb}_{c}")
                    for c in range(2)
                ]
                for k in range(8):
                    for c in range(2):
                        nc.tensor.matmul(
                            psqs[c][:],
                            lhsT=wsb[:, k, c * 128:(c + 1) * 128],
                            rhs=xt_sb[:, k, lo:lo + QB],
                            start=(k == 0),
                            stop=(k == 7),
                            skip_group_check=True,
                        )
                for c in range(2):
                    rope_tail(psqs[c], dest, tb, c)

            def proj_qk_atom(wsb, dest, tb, c):
                """One (proj, c-chunk, token-block): matmul + cast + rope."""
                lo = tb * QB
                psq = ps.tile([128, QB], F32, tag="pp", name=f"psq_{tb}_{c}")
                for k in range(8):
                    nc.tensor.matmul(
                        psq[:],
                        lhsT=wsb[:, k, c * 128:(c + 1) * 128],
                        rhs=xt_sb[:, k, lo:lo + QB],
                        start=(k == 0),
                        stop=(k == 7),
                        skip_group_check=True,
                    )
                rope_tail(psq, dest, tb, c)

            def proj_v_atom(tb, tci):
                """V projection for one 128-token chunk -> vaug (ACT copy)."""
                t128 = tb * (QB // 128) + tci
                psv = ps.tile([128, FG], F32, tag="pp", name=f"psv_{t128}")
                for k in range(8):
                    nc.tensor.matmul(
                        psv[:],
                        lhsT=xt_sb[:, k, t128 * 128:(t128 + 1) * 128],
                        rhs=wv_sb[:, k, :],
                        start=(k == 0),
                        stop=(k == 7),
                        skip_group_check=True,
                    )
                nc.scalar.copy(
                    vaug[:, t128, :].rearrange("p (h c) -> p h c", c=65)[:, :, 0:64],
                    psv[:].rearrange("p (h d) -> p h d", d=64),
                )

            yt_r = yt_d[:].rearrange("(c p) t -> p c t", p=128)

            def oproj_atom(qb, d):
                """Output projection chunk [128 dims, 512 toks] + DMA out."""
                lo = qb * QB
                psy = ps.tile([128, QB], F32, tag="pp", name=f"psy_{qb}_{d}")
                for c2 in range(2):
                    nc.tensor.matmul(
                        psy[:],
                        lhsT=wo_sb[:, c2, d * 128:(d + 1) * 128],
                        rhs=zt_sb[:, c2, lo:lo + QB],
                        start=(c2 == 0),
                        stop=(c2 == 1),
                        skip_group_check=True,
                    )
                yt_sb = yp.tile([128, QB], F16, tag="y")
                if d % 2 == 0:
                    nc.vector.tensor_copy(yt_sb[:], psy[:])
                else:
                    nc.scalar.copy(yt_sb[:], psy[:])
                nc.sync.dma_start(yt_r[:, d, lo:lo + QB], yt_sb[:])

            # ---------- filler machinery ----------
            fillers = deque()

            def fill(n):
                for _ in range(min(n, len(fillers))):
                    fillers.popleft()()

            def proj_unit_fillers(tb):
                return (
                    [lambda c=c: proj_qk_atom(wk_sb, kt_sb, tb, c) for c in range(2)]
                    + [lambda c=c: proj_qk_atom(wq_sb, qt_sb, tb, c) for c in range(2)]
                    + [lambda t=t: proj_v_atom(tb, t) for t in range(QB // 128)]
                )

            def oproj_fillers(qb):
                return [lambda d=d: oproj_atom(qb, d) for d in range(8)]

            # ---------- attention ----------
            def attn(qb, fill_per_slot):
                lo = qb * QB
                J = 4 * qb + 4     # 128-k-blocks in this q-block's causal range
                for pair in range(2):
                    obs = {}
                    for hh in range(2):
                        obs[hh] = ps.tile(
                            [65, QB], F32, tag="ob", name=f"ob_{qb}_{pair}_{hh}"
                        )
                    pts = {}

                    def emit_st(j, pair=pair, pts=pts):
                        qs = max(0, j * 128 - lo)      # local col start
                        st = ps.tile([128, 2 * QB], F32, tag="st")
                        for hh in range(2):
                            nc.tensor.matmul(
                                st[:, hh * QB + qs:hh * QB + QB],
                                lhsT=kt_sb[64 * hh:64 * hh + 64, pair,
                                           j * 128:(j + 1) * 128],
                                rhs=qt_sb[64 * hh:64 * hh + 64, pair,
                                          lo + qs:lo + QB],
                                start=True,
                                stop=True,
                                skip_group_check=True,
                            )
                        pt = ptp.tile([128, 2 * QB], F16, tag="pt")
                        if qs == 0:
                            nc.scalar.activation(pt[:], st[:], Exp, scale=0.125)
                        else:
                            for hh in range(2):
                                nc.scalar.activation(
                                    pt[:, hh * QB + qs:hh * QB + QB],
                                    st[:, hh * QB + qs:hh * QB + QB],
                                    Exp, scale=0.125,
                                )
                        if j * 128 >= lo:  # diagonal block: causal 0/1 mask
                            # GpSimd: idle engine; the diag PV deferral hides
                            # its latency
                            for hh in range(2):
                                nc.gpsimd.tensor_mul(
                                    pt[:, hh * QB + qs:hh * QB + qs + 128],
                                    pt[:, hh * QB + qs:hh * QB + qs + 128],
                                    mask_t[:],
                                )
                        pts[j] = (pt, qs)

                    def emit_pv_nondiag(j, pair=pair, pts=pts):
                        # j == 0 is emitted whole (after the mask if it is
                        # diagonal) so no start=True lands post-accumulation
                        pt, qs = pts[j]
                        diag = j * 128 >= lo
                        c0 = qs if j == 0 else (qs + 128 if diag else qs)
                        if c0 >= QB:
                            return
                        for hh in range(2):
                            hg = pair * 2 + hh
                            nc.tensor.matmul(
                                obs[hh][:, c0:QB],
                                lhsT=vaug[:, j, hg * 65:(hg + 1) * 65],
                                rhs=pt[:, hh * QB + c0:hh * QB + QB],
                                start=(j == 0),
                                stop=False,
                                skip_group_check=True,
                            )

                    def emit_pv_diag(j, pair=pair, pts=pts):
                        pt, qs = pts.pop(j)
                        for hh in range(2):
                            hg = pair * 2 + hh
                            nc.tensor.matmul(
                                obs[hh][:, qs:qs + 128],
                                lhsT=vaug[:, j, hg * 65:(hg + 1) * 65],
                                rhs=pt[:, hh * QB + qs:hh * QB + qs + 128],
                                start=False,
                                stop=True,
                                skip_group_check=True,
                            )

                    emit_st(0)
                    if DEBUG_TAPS and qb == 0 and pair == 0:
                        dbg_pt0 = nc.declare_dram_parameter(
                            "dbg_pt0", [128, 2 * QB], F16, isOutput=True)
                        nc.sync.dma_start(dbg_pt0[:], pts[0][0][:])
                    if J > 1:
                        emit_st(1)
                    for j in range(J):
                        fill(fill_per_slot)
                        if j + 2 < J:
                            emit_st(j + 2)
                        emit_pv_nondiag(j)
                        if j == 0 or j * 128 < lo:
                            pts.pop(j)
                    # deferred diagonal PV chunks (past the DVE mask mul)
                    for j in range(max(1, 4 * qb), J):
                        fill(1 if j % 2 == 0 else 0)
                        emit_pv_diag(j)

                    # normalization, decoupled from PSUM recycling
                    last_pair = (qb == NQB - 1 and pair == 1)
                    for hh in range(2):
                        lrow = npl.tile([1, QB], F32, tag="lrow")
                        nc.vector.tensor_copy(lrow[:], obs[hh][64:65, :])
                        if last_pair:
                            # nothing recycles this PSUM slot: skip the
                            # decoupling copy, read OB directly below
                            ot = obs[hh][0:64, :]
                        else:
                            ot_t = npl.tile([64, QB], F32, tag="ot")
                            nc.vector.tensor_copy(ot_t[:], obs[hh][0:64, :])
                            ot = ot_t[:]
                        rcp = npl.tile([1, QB], F32, tag="rcp")
                        # NB: rcp input must be partition-0 based; feeding a
                        # partition-64 AP returns garbage on hardware.
                        nc.vector.reciprocal_approx_fast(rcp[:], lrow[:])
                        bc = npl.tile([64, QB], F32, tag="bc")
                        nc.gpsimd.partition_broadcast(bc[:], rcp[:])
                        nc.vector.tensor_mul(
                            zt_sb[hh * 64:(hh + 1) * 64, pair, lo:lo + QB],
                            ot,
                            bc[:],
                        )

            # ---------- schedule ----------
            proj_qk_streamed(wk_sb, kt_sb, 0)
            proj_qk_streamed(wq_sb, qt_sb, 0)

            fillers.extend([lambda t=t: proj_v_atom(0, t) for t in range(QB // 128)])
            fillers.extend(proj_unit_fillers(1))
            fillers.extend(proj_unit_fillers(2))
            attn(0, fill_per_slot=3)
            fill(99)

            fillers.extend(proj_unit_fillers(3))
            attn(1, fill_per_slot=1)
            fill(99)

            fillers.extend(oproj_fillers(0))
            attn(2, fill_per_slot=1)
            fill(99)

            fillers.extend(oproj_fillers(1))
            fillers.extend(oproj_fillers(2))
            attn(3, fill_per_slot=1)
            fill(99)

            for d in range(8):
                oproj_atom(3, d)

            if DEBUG_TAPS:
                dbg_kt = nc.declare_dram_parameter("dbg_kt", [128, 2 * S], F16, isOutput=True)
                dbg_qt = nc.declare_dram_parameter("dbg_qt", [128, 2 * S], F16, isOutput=True)
                dbg_zt = nc.declare_dram_parameter("dbg_zt", [128, 2 * S], F16, isOutput=True)
                dbg_wo = nc.declare_dram_parameter("dbg_wo", [128, 2 * DIM], F16, isOutput=True)
                dbg_cos = nc.declare_dram_parameter("dbg_cos", [128, S], F16, isOutput=True)
                dbg_va = nc.declare_dram_parameter("dbg_va", [128, NT * HPG * 65], F16, isOutput=True)
                nc.sync.dma_start(dbg_kt[:].rearrange("p (c t) -> p c t", c=2), kt_sb[:])
                nc.sync.dma_start(dbg_qt[:].rearrange("p (c t) -> p c t", c=2), qt_sb[:])
                nc.sync.dma_start(dbg_zt[:].rearrange("p (c t) -> p c t", c=2), zt_sb[:])
                nc.sync.dma_start(dbg_wo[:].rearrange("p (c d) -> p c d", c=2), wo_sb[:])
                nc.sync.dma_start(dbg_cos[:], cos_t[:])
                nc.sync.dma_start(dbg_va[:].rearrange("p (a b) -> p a b", a=NT), vaug[:])

    nc.compile()
    return nc


def _host_inputs(x, Wq, Wk, Wv, Wo, S):
    """Per-core input maps (host-side sharding + layout prep)."""
    # de-interleave RoPE pairs within each head: (2i, 2i+1) -> (i, i+32)
    perm = np.concatenate([np.arange(0, HEAD_DIM, 2), np.arange(1, HEAD_DIM, 2)])
    rp = (np.arange(HPG)[:, None] * HEAD_DIM + perm[None, :]).reshape(-1)

    half = HEAD_DIM // 2
    inv_freq = THETA ** (-np.arange(half, dtype=np.float64) * 2.0 / HEAD_DIM)
    ang = np.arange(S, dtype=np.float64)[None, :] * inv_freq[:, None]  # [32, S]
    cos32 = np.cos(ang)
    sin32 = np.sin(ang)
    cos128 = np.tile(cos32, (4, 1)).astype(np.float16)
    sins128 = np.concatenate([-sin32, sin32, -sin32, sin32], axis=0).astype(np.float16)
    mask = (np.arange(128)[None, :] >= np.arange(128)[:, None]).astype(np.float16)

    in_maps = []
    for core in range(B * GROUPS):
        b, g = divmod(core, GROUPS)
        sl = slice(g * FG, (g + 1) * FG)
        in_maps.append(
            dict(
                xt=np.ascontiguousarray(x[b].T).astype(np.float16),
                wqt=np.ascontiguousarray(Wq[sl][rp].T).astype(np.float16),
                wkt=np.ascontiguousarray(Wk[sl][rp].T).astype(np.float16),
                wvt=np.ascontiguousarray(Wv[sl].T).astype(np.float16),
                wot=np.ascontiguousarray(Wo[:, sl].T).astype(np.float16),
                cos=cos128,
                sins=sins128,
                mask=mask,
            )
        )
    return in_maps


def _install_ntff_hook():
    """Provide antenv.axon_hooks if the image lacks it (NTFF profiling
    under axon; mirrors trn_agent_boot._ntff_profile_via_ctypes)."""
    try:
        from antenv.axon_hooks import get_axon_ntff_profile_hook  # noqa: F401
        return
    except ImportError:
        pass
    import contextlib
    import ctypes
    import types

    so_path = "/opt/axon/libaxon_pjrt.so"
    if not os.path.exists(so_path):
        return
    lib = ctypes.CDLL(so_path)
    if not hasattr(lib, "axon_start_nrt_profile"):
        return
    lib.axon_start_nrt_profile.argtypes = [
        ctypes.POINTER(ctypes.c_int64),
        ctypes.c_size_t,
    ]
    lib.axon_start_nrt_profile.restype = ctypes.c_int64
    lib.axon_stop_nrt_profile.argtypes = [ctypes.c_char_p]
    lib.axon_stop_nrt_profile.restype = ctypes.c_int64

    @contextlib.contextmanager
    def _hook(output_dir, device_ids):
        import jax

        jax.devices()
        if device_ids:
            ids = (ctypes.c_int64 * len(device_ids))(*device_ids)
            rc = lib.axon_start_nrt_profile(ids, len(device_ids))
        else:
            rc = lib.axon_start_nrt_profile(None, 0)
        if rc != 0:
            raise RuntimeError(f"axon_start_nrt_profile rc={rc}")
        try:
            yield
        finally:
            n = lib.axon_stop_nrt_profile(str(output_dir).encode())
            print(f"profile: {n} file(s) written to {output_dir}")

    mod = types.ModuleType("antenv.axon_hooks")
    _state = {"hook": _hook}
    mod.get_axon_ntff_profile_hook = lambda: _state["hook"]
    mod.set_axon_ntff_profile_hook = lambda h: _state.__setitem__("hook", h)
    import antenv

    antenv.axon_hooks = mod
    sys.modules["antenv.axon_hooks"] = mod


_NC_CACHE = {}


def _get_nc(S):
    if S not in _NC_CACHE:
        _NC_CACHE[S] = build_program(S)
    return _NC_CACHE[S]


def kernel(x, Wq, Wk, Wv, Wo, _trace=False, _tmpdir=None):
    x = np.asarray(x, dtype=np.float32)
    Wq = np.asarray(Wq, dtype=np.float32)
    Wk = np.asarray(Wk, dtype=np.float32)
    Wv = np.asarray(Wv, dtype=np.float32)
    Wo = np.asarray(Wo, dtype=np.float32)
    S = x.shape[1]

    if _trace:
        _install_ntff_hook()
    nc = _get_nc(S)
    in_maps = _host_inputs(x, Wq, Wk, Wv, Wo, S)
    res = run_bass_kernel_spmd(
        nc, in_maps, core_ids=list(range(8)), trace=_trace, tmpdir=_tmpdir
    )
    yts = [res.results[c]["yt"].astype(np.float32) for c in range(8)]
    y = np.stack(
        [sum(yts[b * GROUPS + g] for g in range(GROUPS)).T for b in range(B)]
    ).astype(np.float32)
    if _trace:
        kernel.last_results = res
    return y


# revision 22
# speedup vs baseline: 1.1336x; 1.1336x over previous
"""Trainium2 Bass kernel for causal multi-head attention with RoPE.

Problem: B=2, S=2048, DIM=1024, 16 heads, head_dim=64.
  q = x @ Wq.T ; k = x @ Wk.T ; v = x @ Wv.T        (torch Linear convention)
  q, k = rope(q), rope(k)                            (Llama interleaved pairs)
  y = softmax(causal(q k^T / 8)) v @ Wo.T

Sharding (8 cores): data-parallel over batch (2) x tensor-parallel over
head groups (4 heads per core).  Wq/Wk/Wv row-sharded, Wo column-sharded;
the host sums the 4 partial outputs per batch.

Fused single-stream schedule (v2): projections, attention, and the output
projection are interleaved at fine grain so the PE never waits on the
ScalarE exp stream and ScalarE starts exp'ing ~12us into the kernel
instead of ~70us:

  P(t): Q/K/V projections for one 512-token block (PE) + RoPE (ACT cast,
        DVE mul/rot/add) + V copy to the PV layout (ACT).
  A(q): causal attention for one 512-query block: per 128-k-block j,
        ST (PE, scores^T in PSUM, 2 heads packed in one [128,2,512] tile)
        -> exp (one ScalarE instr over both heads) -> PV accumulate
        (PE, [V|1]-stationary, gives the softmax denominator for free).
        Diagonal-block PV chunks deferred past the DVE causal-mask mul.
        Normalization decoupled from PSUM: OB is copied to SBUF (DVE)
        so the PSUM bank recycles immediately; reciprocal + GpSimd
        partition-broadcast + DVE mul happen off the critical path.
  O(q): output projection of one finished 512-query block + DMA out.

  Emission order (PE executes in order; fillers absorb exp latency):
    warmup-mm, P(0), A(0)[P(1),P(2) interleaved], A(1)[P(3)],
    A(2)[O(0)], A(3)[O(1),O(2)], O(3)

All matmul operands fp16 (accumulation fp32), scores exp'd without max
subtraction (they are O(1) here).  Host bakes: RoPE pair de-interleave
into the W rows (rope swap = +-32 partition copy), cos/sin tables with
signs, causal 0/1 mask, and fp16 transposed layouts.
"""

import os
import sys

sys.path.insert(0, "/opt/trn_rl_repo")

from collections import deque

import numpy as np

import concourse.bass as bass
import concourse.mybir as mybir
import concourse.tile as tile
from concourse import bacc
from concourse.bass_utils import run_bass_kernel_spmd

F16 = mybir.dt.float16
F32 = mybir.dt.float32

DIM = 1024
NUM_HEADS = 16
HEAD_DIM = 64
B = 2
GROUPS = 4                   # head groups (tensor parallel)
HPG = NUM_HEADS // GROUPS    # heads per group = 4
FG = HPG * HEAD_DIM          # features per group = 256
THETA = 10000.0


DEBUG_TAPS = False


def build_program(S=2048):
    from contextlib import ExitStack

    nc = bacc.Bacc(None, target_bir_lowering=False)
    NT = S // 128                 # 128-token blocks (16)
    QB = 512                      # query/token macro-block
    NQB = S // QB                 # 4

    xt_d = nc.declare_dram_parameter("xt", [DIM, S], F16, isOutput=False)
    wq_d = nc.declare_dram_parameter("wqt", [DIM, FG], F16, isOutput=False)
    wk_d = nc.declare_dram_parameter("wkt", [DIM, FG], F16, isOutput=False)
    wv_d = nc.declare_dram_parameter("wvt", [DIM, FG], F16, isOutput=False)
    wo_d = nc.declare_dram_parameter("wot", [FG, DIM], F16, isOutput=False)
    cos_d = nc.declare_dram_parameter("cos", [128, S], F16, isOutput=False)
    sin_d = nc.declare_dram_parameter("sins", [128, S], F16, isOutput=False)
    mask_d = nc.declare_dram_parameter("mask", [128, 128], F16, isOutput=False)
    # fp16 partial output; host upcasts to fp32 before summing the 4 partials
    yt_d = nc.declare_dram_parameter("yt", [DIM, S], F16, isOutput=True)

    Exp = mybir.ActivationFunctionType.Exp

    with tile.TileContext(nc) as tc:
        with ExitStack() as ctx:
            consts = ctx.enter_context(tc.tile_pool(name="consts", bufs=1))
            ps = ctx.enter_context(tc.tile_pool(name="ps", bufs=2, space="PSUM"))
            ptp = ctx.enter_context(tc.tile_pool(name="ptp", bufs=8))
            rp = ctx.enter_context(tc.tile_pool(name="rope", bufs=3))
            npl = ctx.enter_context(tc.tile_pool(name="norm", bufs=4))
            yp = ctx.enter_context(tc.tile_pool(name="ysb", bufs=3))

            wq_sb = consts.tile([128, 8, FG], F16)
            wk_sb = consts.tile([128, 8, FG], F16)
            wv_sb = consts.tile([128, 8, FG], F16)
            wo_sb = consts.tile([128, 2, DIM], F16)
            xt_sb = consts.tile([128, 8, S], F16)
            cos_t = consts.tile([128, S], F16)
            sin_t = consts.tile([128, S], F16)
            mask_t = consts.tile([128, 128], F16)
            qt_sb = consts.tile([128, 2, S], F16)
            kt_sb = consts.tile([128, 2, S], F16)
            zt_sb = consts.tile([128, 2, S], F16)
            vaug = consts.tile([128, NT, HPG * 65], F16)
            warm_sb = consts.tile([128, 512], F16)

            nc.vector.memset(warm_sb[:], 1.0)
            nc.vector.memset(vaug[:], 1.0)

            # ---------- input DMA, need-ordered ----------
            xt_r = xt_d[:].rearrange("(c p) t -> p c t", p=128)

            def dma_xt(tb):
                for k in range(8):
                    nc.sync.dma_start(
                        xt_sb[:, k, tb * QB:(tb + 1) * QB],
                        xt_r[:, k, tb * QB:(tb + 1) * QB],
                    )

            nc.sync.dma_start(wk_sb[:], wk_d[:].rearrange("(c p) f -> p c f", p=128))
            dma_xt(0)
            nc.sync.dma_start(wq_sb[:], wq_d[:].rearrange("(c p) f -> p c f", p=128))
            nc.sync.dma_start(wv_sb[:], wv_d[:].rearrange("(c p) f -> p c f", p=128))
            nc.sync.dma_start(cos_t[:], cos_d[:])
            nc.sync.dma_start(sin_t[:], sin_d[:])
            dma_xt(1)
            nc.sync.dma_start(mask_t[:], mask_d[:])
            dma_xt(2)
            dma_xt(3)
            nc.sync.dma_start(wo_sb[:], wo_d[:].rearrange("(c p) d -> p c d", p=128))

            # ---------- PE p-state warmup (results discarded) ----------
            for i in range(3):
                pw = ps.tile([128, 512], F32, tag="pp", name=f"warm{i}")
                nc.tensor.matmul(
                    pw[:], lhsT=warm_sb[:, 0:128], rhs=warm_sb[:],
                    start=True, stop=True,
                )

            # ---------- unit emitters ----------
            def rope_tail(psq, dest, tb, c):
                lo = tb * QB
                qc = rp.tile([128, QB], F16, tag="qc")
                nc.scalar.copy(qc[:], psq[:])  # fp32->fp16 cast on ACT
                t1 = rp.tile([128, QB], F16, tag="t1")
                nc.vector.tensor_mul(t1[:], qc[:], cos_t[:, lo:lo + QB])
                rot = rp.tile([128, QB], F16, tag="rot")
                for qq in range(4):
                    srcp = (qq ^ 1) * 32
                    nc.vector.tensor_copy(
                        rot[qq * 32:(qq + 1) * 32, :], qc[srcp:srcp + 32, :]
                    )
                t2 = rp.tile([128, QB], F16, tag="t2")
                nc.vector.tensor_mul(t2[:], rot[:], sin_t[:, lo:lo + QB])
                nc.vector.tensor_add(dest[:, c, lo:lo + QB], t1[:], t2[:])

            def proj_qk_streamed(wsb, dest, tb):
                """k-outer / c-inner: first matmul needs only the first xt
                chunk, so the head of the kernel is not DMA-bound."""
                lo = tb * QB
                psqs = [
                    ps.tile([128, QB], F32, tag="pp", name=f"psqs_{tb}_{c}")
                    for c in range(2)
                ]
                for k in range(8):
                    for c in range(2):
                        nc.tensor.matmul(
                            psqs[c][:],
                            lhsT=wsb[:, k, c * 128:(c + 1) * 128],
                            rhs=xt_sb[:, k, lo:lo + QB],
                            start=(k == 0),
                            stop=(k == 7),
                            skip_group_check=True,
                        )
                for c in range(2):
                    rope_tail(psqs[c], dest, tb, c)

            def proj_qk_atom(wsb, dest, tb, c):
                """One (proj, c-chunk, token-block): matmul + cast + rope."""
                lo = tb * QB
                psq = ps.tile([128, QB], F32, tag="pp", name=f"psq_{tb}_{c}")
                for k in range(8):
                    nc.tensor.matmul(
                        psq[:],
                        lhsT=wsb[:, k, c * 128:(c + 1) * 128],
                        rhs=xt_sb[:, k, lo:lo + QB],
                        start=(k == 0),
                        stop=(k == 7),
                        skip_group_check=True,
                    )
                rope_tail(psq, dest, tb, c)

            def proj_v_atom(tb, tci):
                """V projection for one 128-token chunk -> vaug (ACT copy)."""
                t128 = tb * (QB // 128) + tci
                psv = ps.tile([128, FG], F32, tag="pp", name=f"psv_{t128}")
                for k in range(8):
                    nc.tensor.matmul(
                        psv[:],
                        lhsT=xt_sb[:, k, t128 * 128:(t128 + 1) * 128],
                        rhs=wv_sb[:, k, :],
                        start=(k == 0),
                        stop=(k == 7),
                        skip_group_check=True,
                    )
                nc.vector.tensor_copy(
                    vaug[:, t128, :].rearrange("p (h c) -> p h c", c=65)[:, :, 0:64],
                    psv[:].rearrange("p (h d) -> p h d", d=64),
                )

            yt_r = yt_d[:].rearrange("(c p) t -> p c t", p=128)

            def oproj_atom(qb, d):
                """Output projection chunk [128 dims, 512 toks] + DMA out."""
                lo = qb * QB
                psy = ps.tile([128, QB], F32, tag="pp", name=f"psy_{qb}_{d}")
                for c2 in range(2):
                    nc.tensor.matmul(
                        psy[:],
                        lhsT=wo_sb[:, c2, d * 128:(d + 1) * 128],
                        rhs=zt_sb[:, c2, lo:lo + QB],
                        start=(c2 == 0),
                        stop=(c2 == 1),
                        skip_group_check=True,
                    )
                yt_sb = yp.tile([128, QB], F16, tag="y")
                if d % 2 == 0:
                    nc.vector.tensor_copy(yt_sb[:], psy[:])
                else:
                    nc.scalar.copy(yt_sb[:], psy[:])
                nc.sync.dma_start(yt_r[:, d, lo:lo + QB], yt_sb[:])

            # ---------- filler machinery ----------
            fillers = deque()

            def fill(n):
                for _ in range(min(n, len(fillers))):
                    fillers.popleft()()

            def proj_unit_fillers(tb):
                return (
                    [lambda c=c: proj_qk_atom(wk_sb, kt_sb, tb, c) for c in range(2)]
                    + [lambda c=c: proj_qk_atom(wq_sb, qt_sb, tb, c) for c in range(2)]
                    + [lambda t=t: proj_v_atom(tb, t) for t in range(QB // 128)]
                )

            def oproj_fillers(qb):
                return [lambda d=d: oproj_atom(qb, d) for d in range(8)]

            # ---------- attention ----------
            def attn(qb, fill_per_slot):
                lo = qb * QB
                J = 4 * qb + 4     # 128-k-blocks in this q-block's causal range
                for pair in range(2):
                    obs = {}
                    for hh in range(2):
                        obs[hh] = ps.tile(
                            [65, QB], F32, tag="ob", name=f"ob_{qb}_{pair}_{hh}"
                        )
                    pts = {}

                    def emit_st(j, pair=pair, pts=pts):
                        qs = max(0, j * 128 - lo)      # local col start
                        st = ps.tile([128, 2 * QB], F32, tag="st")
                        for hh in range(2):
                            nc.tensor.matmul(
                                st[:, hh * QB + qs:hh * QB + QB],
                                lhsT=kt_sb[64 * hh:64 * hh + 64, pair,
                                           j * 128:(j + 1) * 128],
                                rhs=qt_sb[64 * hh:64 * hh + 64, pair,
                                          lo + qs:lo + QB],
                                start=True,
                                stop=True,
                                skip_group_check=True,
                            )
                        pt = ptp.tile([128, 2 * QB], F16, tag="pt")
                        if qs == 0:
                            nc.scalar.activation(pt[:], st[:], Exp, scale=0.125)
                        else:
                            for hh in range(2):
                                nc.scalar.activation(
                                    pt[:, hh * QB + qs:hh * QB + QB],
                                    st[:, hh * QB + qs:hh * QB + QB],
                                    Exp, scale=0.125,
                                )
                        if j * 128 >= lo:  # diagonal block: causal 0/1 mask
                            for hh in range(2):
                                nc.vector.tensor_mul(
                                    pt[:, hh * QB + qs:hh * QB + qs + 128],
                                    pt[:, hh * QB + qs:hh * QB + qs + 128],
                                    mask_t[:],
                                )
                        pts[j] = (pt, qs)

                    def emit_pv_nondiag(j, pair=pair, pts=pts):
                        # j == 0 is emitted whole (after the mask if it is
                        # diagonal) so no start=True lands post-accumulation
                        pt, qs = pts[j]
                        diag = j * 128 >= lo
                        c0 = qs if j == 0 else (qs + 128 if diag else qs)
                        if c0 >= QB:
                            return
                        for hh in range(2):
                            hg = pair * 2 + hh
                            nc.tensor.matmul(
                                obs[hh][:, c0:QB],
                                lhsT=vaug[:, j, hg * 65:(hg + 1) * 65],
                                rhs=pt[:, hh * QB + c0:hh * QB + QB],
                                start=(j == 0),
                                stop=False,
                                skip_group_check=True,
                            )

                    def emit_pv_diag(j, pair=pair, pts=pts):
                        pt, qs = pts.pop(j)
                        for hh in range(2):
                            hg = pair * 2 + hh
                            nc.tensor.matmul(
                                obs[hh][:, qs:qs + 128],
                                lhsT=vaug[:, j, hg * 65:(hg + 1) * 65],
                                rhs=pt[:, hh * QB + qs:hh * QB + qs + 128],
                                start=False,
                                stop=True,
                                skip_group_check=True,
                            )

                    emit_st(0)
                    if DEBUG_TAPS and qb == 0 and pair == 0:
                        dbg_pt0 = nc.declare_dram_parameter(
                            "dbg_pt0", [128, 2 * QB], F16, isOutput=True)
                        nc.sync.dma_start(dbg_pt0[:], pts[0][0][:])
                    if J > 1:
                        emit_st(1)
                    for j in range(J):
                        fill(fill_per_slot)
                        if j + 2 < J:
                            emit_st(j + 2)
                        emit_pv_nondiag(j)
                        if j == 0 or j * 128 < lo:
                            pts.pop(j)
                    # deferred diagonal PV chunks (past the DVE mask mul)
                    for j in range(max(1, 4 * qb), J):
                        fill(1 if j % 2 == 0 else 0)
                        emit_pv_diag(j)

                    # normalization, decoupled from PSUM recycling
                    last_pair = (qb == NQB - 1 and pair == 1)
                    for hh in range(2):
                        lrow = npl.tile([1, QB], F32, tag="lrow")
                        nc.vector.tensor_copy(lrow[:], obs[hh][64:65, :])
                        if last_pair:
                            # nothing recycles this PSUM slot: skip the
                            # decoupling copy, read OB directly below
                            ot = obs[hh][0:64, :]
                        else:
                            ot_t = npl.tile([64, QB], F32, tag="ot")
                            nc.vector.tensor_copy(ot_t[:], obs[hh][0:64, :])
                            ot = ot_t[:]
                        rcp = npl.tile([1, QB], F32, tag="rcp")
                        # NB: rcp input must be partition-0 based; feeding a
                        # partition-64 AP returns garbage on hardware.
                        nc.vector.reciprocal_approx_fast(rcp[:], lrow[:])
                        bc = npl.tile([64, QB], F32, tag="bc")
                        nc.gpsimd.partition_broadcast(bc[:], rcp[:])
                        nc.vector.tensor_mul(
                            zt_sb[hh * 64:(hh + 1) * 64, pair, lo:lo + QB],
                            ot,
                            bc[:],
                        )

            # ---------- schedule ----------
            proj_qk_streamed(wk_sb, kt_sb, 0)
            proj_qk_streamed(wq_sb, qt_sb, 0)

            fillers.extend([lambda t=t: proj_v_atom(0, t) for t in range(QB // 128)])
            fillers.extend(proj_unit_fillers(1))
            fillers.extend(proj_unit_fillers(2))
            attn(0, fill_per_slot=3)
            fill(99)

            fillers.extend(proj_unit_fillers(3))
            attn(1, fill_per_slot=1)
            fill(99)

            fillers.extend(oproj_fillers(0))
            attn(2, fill_per_slot=1)
            fill(99)

            fillers.extend(oproj_fillers(1))
            fillers.extend(oproj_fillers(2))
            attn(3, fill_per_slot=1)
            fill(99)

            for d in range(8):
                oproj_atom(3, d)

            if DEBUG_TAPS:
                dbg_kt = nc.declare_dram_parameter("dbg_kt", [128, 2 * S], F16, isOutput=True)
                dbg_qt = nc.declare_dram_parameter("dbg_qt", [128, 2 * S], F16, isOutput=True)
                dbg_zt = nc.declare_dram_parameter("dbg_zt", [128, 2 * S], F16, isOutput=True)
                dbg_wo = nc.declare_dram_parameter("dbg_wo", [128, 2 * DIM], F16, isOutput=True)
                dbg_cos = nc.declare_dram_parameter("dbg_cos", [128, S], F16, isOutput=True)
                dbg_va = nc.declare_dram_parameter("dbg_va", [128, NT * HPG * 65], F16, isOutput=True)
                nc.sync.dma_start(dbg_kt[:].rearrange("p (c t) -> p c t", c=2), kt_sb[:])
                nc.sync.dma_start(dbg_qt[:].rearrange("p (c t) -> p c t", c=2), qt_sb[:])
                nc.sync.dma_start(dbg_zt[:].rearrange("p (c t) -> p c t", c=2), zt_sb[:])
                nc.sync.dma_start(dbg_wo[:].rearrange("p (c d) -> p c d", c=2), wo_sb[:])
                nc.sync.dma_start(dbg_cos[:], cos_t[:])
                nc.sync.dma_start(dbg_va[:].rearrange("p (a b) -> p a b", a=NT), vaug[:])

    nc.compile()
    return nc


def _host_inputs(x, Wq, Wk, Wv, Wo, S):
    """Per-core input maps (host-side sharding + layout prep)."""
    # de-interleave RoPE pairs within each head: (2i, 2i+1) -> (i, i+32)
    perm = np.concatenate([np.arange(0, HEAD_DIM, 2), np.arange(1, HEAD_DIM, 2)])
    rp = (np.arange(HPG)[:, None] * HEAD_DIM + perm[None, :]).reshape(-1)

    half = HEAD_DIM // 2
    inv_freq = THETA ** (-np.arange(half, dtype=np.float64) * 2.0 / HEAD_DIM)
    ang = np.arange(S, dtype=np.float64)[None, :] * inv_freq[:, None]  # [32, S]
    cos32 = np.cos(ang)
    sin32 = np.sin(ang)
    cos128 = np.tile(cos32, (4, 1)).astype(np.float16)
    sins128 = np.concatenate([-sin32, sin32, -sin32, sin32], axis=0).astype(np.float16)
    mask = (np.arange(128)[None, :] >= np.arange(128)[:, None]).astype(np.float16)

    in_maps = []
    for core in range(B * GROUPS):
        b, g = divmod(core, GROUPS)
        sl = slice(g * FG, (g + 1) * FG)
        in_maps.append(
            dict(
                xt=np.ascontiguousarray(x[b].T).astype(np.float16),
                wqt=np.ascontiguousarray(Wq[sl][rp].T).astype(np.float16),
                wkt=np.ascontiguousarray(Wk[sl][rp].T).astype(np.float16),
                wvt=np.ascontiguousarray(Wv[sl].T).astype(np.float16),
                wot=np.ascontiguousarray(Wo[:, sl].T).astype(np.float16),
                cos=cos128,
                sins=sins128,
                mask=mask,
            )
        )
    return in_maps


def _install_ntff_hook():
    """Provide antenv.axon_hooks if the image lacks it (NTFF profiling
    under axon; mirrors trn_agent_boot._ntff_profile_via_ctypes)."""
    try:
        from antenv.axon_hooks import get_axon_ntff_profile_hook  # noqa: F401
        return
    except ImportError:
        pass
    import contextlib
    import ctypes
    import types

    so_path = "/opt/axon/libaxon_pjrt.so"
    if not os.path.exists(so_path):
        return
    lib = ctypes.CDLL(so_path)
    if not hasattr(lib, "axon_start_nrt_profile"):
        return
    lib.axon_start_nrt_profile.argtypes = [
        ctypes.POINTER(ctypes.c_int64),
        ctypes.c_size_t,
    ]
    lib.axon_start_nrt_profile.restype = ctypes.c_int64
    lib.axon_stop_nrt_profile.argtypes = [ctypes.c_char_p]
    lib.axon_stop_nrt_profile.restype = ctypes.c_int64

    @contextlib.contextmanager
    def _hook(output_dir, device_ids):
        import jax

        jax.devices()
        if device_ids:
            ids = (ctypes.c_int64 * len(device_ids))(*device_ids)
            rc = lib.axon_start_nrt_profile(ids, len(device_ids))
        else:
            rc = lib.axon_start_nrt_profile(None, 0)
        if rc != 0:
            raise RuntimeError(f"axon_start_nrt_profile rc={rc}")
        try:
            yield
        finally:
            n = lib.axon_stop_nrt_profile(str(output_dir).encode())
            print(f"profile: {n} file(s) written to {output_dir}")

    mod = types.ModuleType("antenv.axon_hooks")
    _state = {"hook": _hook}
    mod.get_axon_ntff_profile_hook = lambda: _state["hook"]
    mod.set_axon_ntff_profile_hook = lambda h: _state.__setitem__("hook", h)
    import antenv

    antenv.axon_hooks = mod
    sys.modules["antenv.axon_hooks"] = mod


_NC_CACHE = {}


def _get_nc(S):
    if S not in _NC_CACHE:
        _NC_CACHE[S] = build_program(S)
    return _NC_CACHE[S]


def kernel(x, Wq, Wk, Wv, Wo, _trace=False, _tmpdir=None):
    x = np.asarray(x, dtype=np.float32)
    Wq = np.asarray(Wq, dtype=np.float32)
    Wk = np.asarray(Wk, dtype=np.float32)
    Wv = np.asarray(Wv, dtype=np.float32)
    Wo = np.asarray(Wo, dtype=np.float32)
    S = x.shape[1]

    if _trace:
        _install_ntff_hook()
    nc = _get_nc(S)
    in_maps = _host_inputs(x, Wq, Wk, Wv, Wo, S)
    res = run_bass_kernel_spmd(
        nc, in_maps, core_ids=list(range(8)), trace=_trace, tmpdir=_tmpdir
    )
    yts = [res.results[c]["yt"].astype(np.float32) for c in range(8)]
    y = np.stack(
        [sum(yts[b * GROUPS + g] for g in range(GROUPS)).T for b in range(B)]
    ).astype(np.float32)
    if _trace:
        kernel.last_results = res
    return y


# revision 23
# speedup vs baseline: 1.3869x; 1.2234x over previous
"""Trainium2 Bass kernel for causal multi-head attention with RoPE.

Problem: B=2, S=2048, DIM=1024, 16 heads, head_dim=64.
  q = x @ Wq.T ; k = x @ Wk.T ; v = x @ Wv.T        (torch Linear convention)
  q, k = rope(q), rope(k)                            (Llama interleaved pairs)
  y = softmax(causal(q k^T / 8)) v @ Wo.T

Sharding (8 cores): data-parallel over batch (2) x tensor-parallel over
head groups (4 heads per core).  Wq/Wk/Wv row-sharded, Wo column-sharded;
the host sums the 4 partial outputs per batch.

Fused single-stream schedule (v2): projections, attention, and the output
projection are interleaved at fine grain so the PE never waits on the
ScalarE exp stream and ScalarE starts exp'ing ~12us into the kernel
instead of ~70us:

  P(t): Q/K/V projections for one 512-token block (PE) + RoPE (ACT cast,
        DVE mul/rot/add) + V copy to the PV layout (ACT).
  A(q): causal attention for one 512-query block: per 128-k-block j,
        ST (PE, scores^T in PSUM, 2 heads packed in one [128,2,512] tile)
        -> exp (one ScalarE instr over both heads) -> PV accumulate
        (PE, [V|1]-stationary, gives the softmax denominator for free).
        Diagonal-block PV chunks deferred past the DVE causal-mask mul.
        Normalization decoupled from PSUM: OB is copied to SBUF (DVE)
        so the PSUM bank recycles immediately; reciprocal + GpSimd
        partition-broadcast + DVE mul happen off the critical path.
  O(q): output projection of one finished 512-query block + DMA out.

  Emission order (PE executes in order; fillers absorb exp latency):
    warmup-mm, P(0), A(0)[P(1),P(2) interleaved], A(1)[P(3)],
    A(2)[O(0)], A(3)[O(1),O(2)], O(3)

All matmul operands fp16 (accumulation fp32), scores exp'd without max
subtraction (they are O(1) here).  Host bakes: RoPE pair de-interleave
into the W rows (rope swap = +-32 partition copy), cos/sin tables with
signs, causal 0/1 mask, and fp16 transposed layouts.
"""

import os
import sys

sys.path.insert(0, "/opt/trn_rl_repo")

from collections import deque

import numpy as np

import concourse.bass as bass
import concourse.mybir as mybir
import concourse.tile as tile
from concourse import bacc
from concourse.bass_utils import run_bass_kernel_spmd

F16 = mybir.dt.float16
F32 = mybir.dt.float32

DIM = 1024
NUM_HEADS = 16
HEAD_DIM = 64
B = 2
GROUPS = 4                   # head groups (tensor parallel)
HPG = NUM_HEADS // GROUPS    # heads per group = 4
FG = HPG * HEAD_DIM          # features per group = 256
THETA = 10000.0


DEBUG_TAPS = False


def build_program(S=2048):
    from contextlib import ExitStack

    nc = bacc.Bacc(None, target_bir_lowering=False)
    NT = S // 128                 # 128-token blocks (16)
    QB = 512                      # query/token macro-block
    NQB = S // QB                 # 4

    xt_d = nc.declare_dram_parameter("xt", [DIM, S], F16, isOutput=False)
    wq_d = nc.declare_dram_parameter("wqt", [DIM, FG], F16, isOutput=False)
    wk_d = nc.declare_dram_parameter("wkt", [DIM, FG], F16, isOutput=False)
    wv_d = nc.declare_dram_parameter("wvt", [DIM, FG], F16, isOutput=False)
    wo_d = nc.declare_dram_parameter("wot", [FG, DIM], F16, isOutput=False)
    cos_d = nc.declare_dram_parameter("cos", [128, S], F16, isOutput=False)
    sin_d = nc.declare_dram_parameter("sins", [128, S], F16, isOutput=False)
    mask_d = nc.declare_dram_parameter("mask", [128, 128], F16, isOutput=False)
    # fp16 partial output; host upcasts to fp32 before summing the 4 partials
    yt_d = nc.declare_dram_parameter("yt", [DIM, S], F16, isOutput=True)

    Exp = mybir.ActivationFunctionType.Exp

    with tile.TileContext(nc) as tc:
        with ExitStack() as ctx:
            consts = ctx.enter_context(tc.tile_pool(name="consts", bufs=1))
            ps = ctx.enter_context(tc.tile_pool(name="ps", bufs=2, space="PSUM"))
            ptp = ctx.enter_context(tc.tile_pool(name="ptp", bufs=8))
            rp = ctx.enter_context(tc.tile_pool(name="rope", bufs=3))
            npl = ctx.enter_context(tc.tile_pool(name="norm", bufs=4))
            yp = ctx.enter_context(tc.tile_pool(name="ysb", bufs=3))

            wq_sb = consts.tile([128, 8, FG], F16)
            wk_sb = consts.tile([128, 8, FG], F16)
            wv_sb = consts.tile([128, 8, FG], F16)
            wo_sb = consts.tile([128, 2, DIM], F16)
            xt_sb = consts.tile([128, 8, S], F16)
            cos_t = consts.tile([128, S], F16)
            sin_t = consts.tile([128, S], F16)
            mask_t = consts.tile([128, 128], F16)
            qt_sb = consts.tile([128, 2, S], F16)
            kt_sb = consts.tile([128, 2, S], F16)
            zt_sb = consts.tile([128, 2, S], F16)
            vaug = consts.tile([128, NT, HPG * 65], F16)
            warm_sb = consts.tile([128, 512], F16)

            nc.vector.memset(warm_sb[:], 1.0)
            nc.vector.memset(vaug[:], 1.0)

            # ---------- input DMA, need-ordered ----------
            xt_r = xt_d[:].rearrange("(c p) t -> p c t", p=128)

            def dma_xt(tb):
                for k in range(8):
                    nc.sync.dma_start(
                        xt_sb[:, k, tb * QB:(tb + 1) * QB],
                        xt_r[:, k, tb * QB:(tb + 1) * QB],
                    )

            nc.sync.dma_start(wk_sb[:], wk_d[:].rearrange("(c p) f -> p c f", p=128))
            dma_xt(0)
            nc.sync.dma_start(wq_sb[:], wq_d[:].rearrange("(c p) f -> p c f", p=128))
            nc.sync.dma_start(wv_sb[:], wv_d[:].rearrange("(c p) f -> p c f", p=128))
            nc.sync.dma_start(cos_t[:], cos_d[:])
            nc.sync.dma_start(sin_t[:], sin_d[:])
            dma_xt(1)
            nc.sync.dma_start(mask_t[:], mask_d[:])
            dma_xt(2)
            dma_xt(3)
            nc.sync.dma_start(wo_sb[:], wo_d[:].rearrange("(c p) d -> p c d", p=128))

            # ---------- PE p-state warmup (results discarded) ----------
            for i in range(6):
                pw = ps.tile([128, 512], F32, tag="pp", name=f"warm{i}")
                nc.tensor.matmul(
                    pw[:], lhsT=warm_sb[:, 0:128], rhs=warm_sb[:],
                    start=True, stop=True,
                )

            # ---------- unit emitters ----------
            def proj_qk_atom(wsb, dest, tb, c):
                """One (proj, c-chunk, token-block): matmul + cast + rope."""
                lo = tb * QB
                psq = ps.tile([128, QB], F32, tag="pp", name=f"psq_{tb}_{c}")
                for k in range(8):
                    nc.tensor.matmul(
                        psq[:],
                        lhsT=wsb[:, k, c * 128:(c + 1) * 128],
                        rhs=xt_sb[:, k, lo:lo + QB],
                        start=(k == 0),
                        stop=(k == 7),
                        skip_group_check=True,
                    )
                qc = rp.tile([128, QB], F16, tag="qc")
                nc.scalar.copy(qc[:], psq[:])  # fp32->fp16 cast on ACT
                t1 = rp.tile([128, QB], F16, tag="t1")
                nc.vector.tensor_mul(t1[:], qc[:], cos_t[:, lo:lo + QB])
                rot = rp.tile([128, QB], F16, tag="rot")
                for qq in range(4):
                    srcp = (qq ^ 1) * 32
                    nc.vector.tensor_copy(
                        rot[qq * 32:(qq + 1) * 32, :], qc[srcp:srcp + 32, :]
                    )
                t2 = rp.tile([128, QB], F16, tag="t2")
                nc.vector.tensor_mul(t2[:], rot[:], sin_t[:, lo:lo + QB])
                nc.vector.tensor_add(dest[:, c, lo:lo + QB], t1[:], t2[:])

            def proj_v_atom(tb, tci):
                """V projection for one 128-token chunk -> vaug (ACT copy)."""
                t128 = tb * (QB // 128) + tci
                psv = ps.tile([128, FG], F32, tag="pp", name=f"psv_{t128}")
                for k in range(8):
                    nc.tensor.matmul(
                        psv[:],
                        lhsT=xt_sb[:, k, t128 * 128:(t128 + 1) * 128],
                        rhs=wv_sb[:, k, :],
                        start=(k == 0),
                        stop=(k == 7),
                        skip_group_check=True,
                    )
                nc.vector.tensor_copy(
                    vaug[:, t128, :].rearrange("p (h c) -> p h c", c=65)[:, :, 0:64],
                    psv[:].rearrange("p (h d) -> p h d", d=64),
                )

            yt_r = yt_d[:].rearrange("(c p) t -> p c t", p=128)

            def oproj_atom(qb, d):
                """Output projection chunk [128 dims, 512 toks] + DMA out."""
                lo = qb * QB
                psy = ps.tile([128, QB], F32, tag="pp", name=f"psy_{qb}_{d}")
                for c2 in range(2):
                    nc.tensor.matmul(
                        psy[:],
                        lhsT=wo_sb[:, c2, d * 128:(d + 1) * 128],
                        rhs=zt_sb[:, c2, lo:lo + QB],
                        start=(c2 == 0),
                        stop=(c2 == 1),
                        skip_group_check=True,
                    )
                yt_sb = yp.tile([128, QB], F16, tag="y")
                nc.vector.tensor_copy(yt_sb[:], psy[:])
                nc.sync.dma_start(yt_r[:, d, lo:lo + QB], yt_sb[:])

            # ---------- filler machinery ----------
            fillers = deque()

            def fill(n):
                for _ in range(min(n, len(fillers))):
                    fillers.popleft()()

            def proj_unit_fillers(tb):
                return (
                    [lambda c=c: proj_qk_atom(wk_sb, kt_sb, tb, c) for c in range(2)]
                    + [lambda c=c: proj_qk_atom(wq_sb, qt_sb, tb, c) for c in range(2)]
                    + [lambda t=t: proj_v_atom(tb, t) for t in range(QB // 128)]
                )

            def oproj_fillers(qb):
                return [lambda d=d: oproj_atom(qb, d) for d in range(8)]

            # ---------- attention ----------
            def attn(qb, fill_per_slot):
                lo = qb * QB
                J = 4 * qb + 4     # 128-k-blocks in this q-block's causal range
                for pair in range(2):
                    obs = {}
                    for hh in range(2):
                        obs[hh] = ps.tile(
                            [65, QB], F32, tag="ob", name=f"ob_{qb}_{pair}_{hh}"
                        )
                    pts = {}

                    def emit_st(j, pair=pair, pts=pts):
                        qs = max(0, j * 128 - lo)      # local col start
                        st = ps.tile([128, 2 * QB], F32, tag="st")
                        for hh in range(2):
                            nc.tensor.matmul(
                                st[:, hh * QB + qs:hh * QB + QB],
                                lhsT=kt_sb[64 * hh:64 * hh + 64, pair,
                                           j * 128:(j + 1) * 128],
                                rhs=qt_sb[64 * hh:64 * hh + 64, pair,
                                          lo + qs:lo + QB],
                                start=True,
                                stop=True,
                                skip_group_check=True,
                            )
                        pt = ptp.tile([128, 2 * QB], F16, tag="pt")
                        if qs == 0:
                            nc.scalar.activation(pt[:], st[:], Exp, scale=0.125)
                        else:
                            for hh in range(2):
                                nc.scalar.activation(
                                    pt[:, hh * QB + qs:hh * QB + QB],
                                    st[:, hh * QB + qs:hh * QB + QB],
                                    Exp, scale=0.125,
                                )
                        if j * 128 >= lo:  # diagonal block: causal 0/1 mask
                            for hh in range(2):
                                nc.vector.tensor_mul(
                                    pt[:, hh * QB + qs:hh * QB + qs + 128],
                                    pt[:, hh * QB + qs:hh * QB + qs + 128],
                                    mask_t[:],
                                )
                        pts[j] = (pt, qs)

                    def emit_pv_nondiag(j, pair=pair, pts=pts):
                        # j == 0 is emitted whole (after the mask if it is
                        # diagonal) so no start=True lands post-accumulation
                        pt, qs = pts[j]
                        diag = j * 128 >= lo
                        c0 = qs if j == 0 else (qs + 128 if diag else qs)
                        if c0 >= QB:
                            return
                        for hh in range(2):
                            hg = pair * 2 + hh
                            nc.tensor.matmul(
                                obs[hh][:, c0:QB],
                                lhsT=vaug[:, j, hg * 65:(hg + 1) * 65],
                                rhs=pt[:, hh * QB + c0:hh * QB + QB],
                                start=(j == 0),
                                stop=False,
                                skip_group_check=True,
                            )

                    def emit_pv_diag(j, pair=pair, pts=pts):
                        pt, qs = pts.pop(j)
                        for hh in range(2):
                            hg = pair * 2 + hh
                            nc.tensor.matmul(
                                obs[hh][:, qs:qs + 128],
                                lhsT=vaug[:, j, hg * 65:(hg + 1) * 65],
                                rhs=pt[:, hh * QB + qs:hh * QB + qs + 128],
                                start=False,
                                stop=True,
                                skip_group_check=True,
                            )

                    emit_st(0)
                    if DEBUG_TAPS and qb == 0 and pair == 0:
                        dbg_pt0 = nc.declare_dram_parameter(
                            "dbg_pt0", [128, 2 * QB], F16, isOutput=True)
                        nc.sync.dma_start(dbg_pt0[:], pts[0][0][:])
                    if J > 1:
                        emit_st(1)
                    for j in range(J):
                        fill(fill_per_slot)
                        if j + 2 < J:
                            emit_st(j + 2)
                        emit_pv_nondiag(j)
                        if j == 0 or j * 128 < lo:
                            pts.pop(j)
                    # deferred diagonal PV chunks (past the DVE mask mul)
                    for j in range(max(1, 4 * qb), J):
                        fill(1 if j % 2 == 0 else 0)
                        emit_pv_diag(j)

                    # normalization, decoupled from PSUM recycling
                    for hh in range(2):
                        ot = npl.tile([64, QB], F32, tag="ot")
                        nc.vector.tensor_copy(ot[:], obs[hh][0:64, :])
                        lrow = npl.tile([1, QB], F32, tag="lrow")
                        nc.vector.tensor_copy(lrow[:], obs[hh][64:65, :])
                        rcp = npl.tile([1, QB], F32, tag="rcp")
                        # NB: rcp input must be partition-0 based; feeding a
                        # partition-64 AP returns garbage on hardware.
                        nc.vector.reciprocal_approx_fast(rcp[:], lrow[:])
                        bc = npl.tile([64, QB], F32, tag="bc")
                        nc.gpsimd.partition_broadcast(bc[:], rcp[:])
                        if DEBUG_TAPS and qb == 0 and pair == 0 and hh == 0:
                            dbg_ob0 = nc.declare_dram_parameter(
                                "dbg_ob0", [64, QB], F32, isOutput=True)
                            nc.sync.dma_start(dbg_ob0[:], ot[:])
                            dbg_bc0 = nc.declare_dram_parameter(
                                "dbg_bc0", [64, QB], F32, isOutput=True)
                            nc.sync.dma_start(dbg_bc0[:], bc[:])
                        nc.vector.tensor_mul(
                            zt_sb[hh * 64:(hh + 1) * 64, pair, lo:lo + QB],
                            ot[:],
                            bc[:],
                        )

            # ---------- schedule ----------
            for c in range(2):
                proj_qk_atom(wk_sb, kt_sb, 0, c)
            for c in range(2):
                proj_qk_atom(wq_sb, qt_sb, 0, c)
            for t in range(QB // 128):
                proj_v_atom(0, t)

            fillers.extend(proj_unit_fillers(1))
            fillers.extend(proj_unit_fillers(2))
            attn(0, fill_per_slot=2)
            fill(99)

            fillers.extend(proj_unit_fillers(3))
            attn(1, fill_per_slot=1)
            fill(99)

            fillers.extend(oproj_fillers(0))
            attn(2, fill_per_slot=1)
            fill(99)

            fillers.extend(oproj_fillers(1))
            fillers.extend(oproj_fillers(2))
            attn(3, fill_per_slot=1)
            fill(99)

            for d in range(8):
                oproj_atom(3, d)

            if DEBUG_TAPS:
                dbg_kt = nc.declare_dram_parameter("dbg_kt", [128, 2 * S], F16, isOutput=True)
                dbg_qt = nc.declare_dram_parameter("dbg_qt", [128, 2 * S], F16, isOutput=True)
                dbg_zt = nc.declare_dram_parameter("dbg_zt", [128, 2 * S], F16, isOutput=True)
                dbg_wo = nc.declare_dram_parameter("dbg_wo", [128, 2 * DIM], F16, isOutput=True)
                dbg_cos = nc.declare_dram_parameter("dbg_cos", [128, S], F16, isOutput=True)
                dbg_va = nc.declare_dram_parameter("dbg_va", [128, NT * HPG * 65], F16, isOutput=True)
                nc.sync.dma_start(dbg_kt[:].rearrange("p (c t) -> p c t", c=2), kt_sb[:])
                nc.sync.dma_start(dbg_qt[:].rearrange("p (c t) -> p c t", c=2), qt_sb[:])
                nc.sync.dma_start(dbg_zt[:].rearrange("p (c t) -> p c t", c=2), zt_sb[:])
                nc.sync.dma_start(dbg_wo[:].rearrange("p (c d) -> p c d", c=2), wo_sb[:])
                nc.sync.dma_start(dbg_cos[:], cos_t[:])
                nc.sync.dma_start(dbg_va[:].rearrange("p (a b) -> p a b", a=NT), vaug[:])

    nc.compile()
    return nc


def _host_inputs(x, Wq, Wk, Wv, Wo, S):
    """Per-core input maps (host-side sharding + layout prep)."""
    # de-interleave RoPE pairs within each head: (2i, 2i+1) -> (i, i+32)
    perm = np.concatenate([np.arange(0, HEAD_DIM, 2), np.arange(1, HEAD_DIM, 2)])
    rp = (np.arange(HPG)[:, None] * HEAD_DIM + perm[None, :]).reshape(-1)

    half = HEAD_DIM // 2
    inv_freq = THETA ** (-np.arange(half, dtype=np.float64) * 2.0 / HEAD_DIM)
    ang = np.arange(S, dtype=np.float64)[None, :] * inv_freq[:, None]  # [32, S]
    cos32 = np.cos(ang)
    sin32 = np.sin(ang)
    cos128 = np.tile(cos32, (4, 1)).astype(np.float16)
    sins128 = np.concatenate([-sin32, sin32, -sin32, sin32], axis=0).astype(np.float16)
    mask = (np.arange(128)[None, :] >= np.arange(128)[:, None]).astype(np.float16)

    in_maps = []
    for core in range(B * GROUPS):
        b, g = divmod(core, GROUPS)
        sl = slice(g * FG, (g + 1) * FG)
        in_maps.append(
            dict(
                xt=np.ascontiguousarray(x[b].T).astype(np.float16),
                wqt=np.ascontiguousarray(Wq[sl][rp].T).astype(np.float16),
                wkt=np.ascontiguousarray(Wk[sl][rp].T).astype(np.float16),
                wvt=np.ascontiguousarray(Wv[sl].T).astype(np.float16),
                wot=np.ascontiguousarray(Wo[:, sl].T).astype(np.float16),
                cos=cos128,
                sins=sins128,
                mask=mask,
            )
        )
    return in_maps


def _install_ntff_hook():
    """Provide antenv.axon_hooks if the image lacks it (NTFF profiling
    under axon; mirrors trn_agent_boot._ntff_profile_via_ctypes)."""
    try:
        from antenv.axon_hooks import get_axon_ntff_profile_hook  # noqa: F401
        return
    except ImportError:
        pass
    import contextlib
    import ctypes
    import types

    so_path = "/opt/axon/libaxon_pjrt.so"
    if not os.path.exists(so_path):
        return
    lib = ctypes.CDLL(so_path)
    if not hasattr(lib, "axon_start_nrt_profile"):
        return
    lib.axon_start_nrt_profile.argtypes = [
        ctypes.POINTER(ctypes.c_int64),
        ctypes.c_size_t,
    ]
    lib.axon_start_nrt_profile.restype = ctypes.c_int64
    lib.axon_stop_nrt_profile.argtypes = [ctypes.c_char_p]
    lib.axon_stop_nrt_profile.restype = ctypes.c_int64

    @contextlib.contextmanager
    def _hook(output_dir, device_ids):
        import jax

        jax.devices()
        if device_ids:
            ids = (ctypes.c_int64 * len(device_ids))(*device_ids)
            rc = lib.axon_start_nrt_profile(ids, len(device_ids))
        else:
            rc = lib.axon_start_nrt_profile(None, 0)
        if rc != 0:
            raise RuntimeError(f"axon_start_nrt_profile rc={rc}")
        try:
            yield
        finally:
            n = lib.axon_stop_nrt_profile(str(output_dir).encode())
            print(f"profile: {n} file(s) written to {output_dir}")

    mod = types.ModuleType("antenv.axon_hooks")
    _state = {"hook": _hook}
    mod.get_axon_ntff_profile_hook = lambda: _state["hook"]
    mod.set_axon_ntff_profile_hook = lambda h: _state.__setitem__("hook", h)
    import antenv

    antenv.axon_hooks = mod
    sys.modules["antenv.axon_hooks"] = mod


_NC_CACHE = {}


def _get_nc(S):
    if S not in _NC_CACHE:
        _NC_CACHE[S] = build_program(S)
    return _NC_CACHE[S]


def kernel(x, Wq, Wk, Wv, Wo, _trace=False, _tmpdir=None):
    x = np.asarray(x, dtype=np.float32)
    Wq = np.asarray(Wq, dtype=np.float32)
    Wk = np.asarray(Wk, dtype=np.float32)
    Wv = np.asarray(Wv, dtype=np.float32)
    Wo = np.asarray(Wo, dtype=np.float32)
    S = x.shape[1]

    if _trace:
        _install_ntff_hook()
    nc = _get_nc(S)
    in_maps = _host_inputs(x, Wq, Wk, Wv, Wo, S)
    res = run_bass_kernel_spmd(
        nc, in_maps, core_ids=list(range(8)), trace=_trace, tmpdir=_tmpdir
    )
    yts = [res.results[c]["yt"].astype(np.float32) for c in range(8)]
    y = np.stack(
        [sum(yts[b * GROUPS + g] for g in range(GROUPS)).T for b in range(B)]
    ).astype(np.float32)
    if _trace:
        kernel.last_results = res
    return y
